# revision 21
# baseline (speedup 1.0000x reference)
"""NPMLPHead sampling kernel for Trainium2 (Bass/Tile), SPMD over 8 cores.

Strategy (data-parallel over batch, per sharding hint): B=16 -> 2 images/core,
full (tiny) MLP weights replicated per core.

v2 redesign around the measured bottleneck (per-dma_start sequencer time on the
DMA-issue queues, ~0.9-1.6us each for the 256 patch gathers):
  - L0/L1 (sparse: 128 of 16K/4K positions): gather patch column-vectors with
    one strided DMA per patch ([cp, (b ch)] elements; land transposed in SBUF).
    The 256 gather DMAs are cost-balanced across FOUR issue queues: both HWDGE
    rings (sync/scalar), the SWDGE ring (gpsimd), and the DVE sequencer (HWDGE
    policy extended -- the RTL is engine-agnostic).  Patches are issued in
    sorted-position order for HBM row locality.
  - L2 (dense-ish: 128 of 1024): stream the whole shard and use associativity
    y = S^T ((relu(W1^T T))^T W2); layer-1/2 over ALL positions keeps the
    contraction dim on partitions in the native [C, HW] layout, and the
    one-hot select is a single PE matmul chain.  The whole L2 compute is
    WOVEN between the gather DMAs (PE runs it under the gather phase; its
    relu/copy work is interleaved into the scalar queue's instruction stream
    so nothing stalls).
  - L1 gathers go first so the L1 MLP (the bigger tail) overlaps the L0
    gather phase; elementwise tails ride DVE.
  - All matmuls in float32r (single-pass fp32; ~tf32 rounding).
"""

import sys

sys.path.insert(0, "/opt/trn_rl_repo")

import numpy as np

B = 16
N_CORES = 8
B_LOC = B // N_CORES  # 2
P = 128  # NUM_PATCHES
NCD = 256  # MLP width
LEVELS = [(256, 128), (512, 64), (1024, 32)]  # (C, H) per level
EPS = 1e-7

USE_DVE_DMA = False  # compiler supports only the two HWDGE queues + SWDGE

# per-queue per-job issue-cost model (ns), used only for load balancing
COST = {
    "sync": {1: 1106, 0: 865},
    "scalar": {1: 1106, 0: 865},
    "vector": {1: 1146, 0: 905},
    "gpsimd": {1: 1636, 0: 1462},
}
BASE = {"sync": 6000, "scalar": 23800, "vector": 2500, "gpsimd": 0}


def _build(idx_vals):
    import concourse.bass as bass
    import concourse.tile as tile
    from concourse import bacc, mybir

    f32 = mybir.dt.float32
    fr = mybir.dt.float32r
    AF = mybir.ActivationFunctionType

    nc = bacc.Bacc(None)
    if USE_DVE_DMA:
        nc.hwdge_engines.add(mybir.EngineType.DVE)

    feats, w1s, b1s, w2s, b2s = [], [], [], [], []
    for l, (C, H) in enumerate(LEVELS):
        feats.append(
            nc.dram_tensor(f"feat{l}", [B_LOC, C, H, H], fr, kind="ExternalInput")
        )
        w1s.append(nc.dram_tensor(f"w1_{l}", [C, NCD], fr, kind="ExternalInput"))
        b1s.append(nc.dram_tensor(f"b1_{l}", [NCD], fr, kind="ExternalInput"))
        w2s.append(nc.dram_tensor(f"w2_{l}", [NCD, NCD], fr, kind="ExternalInput"))
        b2s.append(nc.dram_tensor(f"b2_{l}", [NCD], fr, kind="ExternalInput"))
    oh = nc.dram_tensor("oh2", [8, 128, P], fr, kind="ExternalInput")
    out = nc.dram_tensor("out", [3, B_LOC, P, NCD], f32, kind="ExternalOutput")

    C2, H2 = LEVELS[2]
    HW2 = H2 * H2  # 1024
    NCH2 = C2 // 128  # 8
    QC2 = HW2 // 128  # 8 q-chunks

    with tile.TileContext(nc) as tc:
        with (
            tc.tile_pool(name="consts", bufs=1) as consts,
            tc.tile_pool(name="xt", bufs=1) as xtp,
            tc.tile_pool(name="work", bufs=4) as work,
            tc.tile_pool(name="psum", bufs=2, space=bass.MemorySpace.PSUM) as psum,
            tc.tile_pool(name="psum1", bufs=1, space=bass.MemorySpace.PSUM) as psum1,
        ):
            QS = {
                "sync": nc.sync,
                "scalar": nc.scalar,
                "vector": nc.vector,
                "gpsimd": nc.gpsimd,
            }
            qnames = ["sync", "scalar", "gpsimd"] + (
                ["vector"] if USE_DVE_DMA else []
            )

            ones_f = consts.tile([1, 512], f32, tag="ones_f")
            nc.vector.memset(ones_f[:], 1.0)
            ones = consts.tile([1, 512], fr, tag="ones")
            nc.vector.tensor_copy(ones[:], ones_f[:])

            # --- early loads, alternating the two HWDGE rings ---
            w1_sb, w2_sb, b1_sb, b2_sb, xts = [], [], [], [], []
            for l, (C, H) in enumerate(LEVELS):
                n_ch = C // 128
                t = consts.tile([128, n_ch * NCD], fr, tag=f"w1_{l}", name=f"w1sb{l}")
                (nc.scalar if l == 2 else nc.sync).dma_start(
                    t[:].rearrange("cp (ch n) -> cp ch n", n=NCD),
                    w1s[l][:].rearrange("(ch cp) n -> cp ch n", cp=128),
                )
                w1_sb.append(t)
                t = consts.tile([128, 2 * NCD], fr, tag=f"w2_{l}", name=f"w2sb{l}")
                (nc.scalar if l == 2 else nc.sync).dma_start(
                    t[:].rearrange("cp (ch n) -> cp ch n", n=NCD),
                    w2s[l][:].rearrange("(ch cp) n -> cp ch n", cp=128),
                )
                w2_sb.append(t)
                t = consts.tile([1, NCD], fr, tag=f"b1_{l}", name=f"b1sb{l}")
                nc.sync.dma_start(t[:], b1s[l][:].rearrange("(o n) -> o n", o=1))
                b1_sb.append(t)
                t = consts.tile([1, NCD], fr, tag=f"b2_{l}", name=f"b2sb{l}")
                nc.sync.dma_start(t[:], b2s[l][:].rearrange("(o n) -> o n", o=1))
                b2_sb.append(t)
                if l < 2:
                    xts.append(
                        xtp.tile(
                            [128, B_LOC * n_ch * P], fr, tag=f"xt_{l}", name=f"xt{l}"
                        )
                    )

            oh_sb = consts.tile([128, QC2 * P], fr, tag="oh2")
            nc.scalar.dma_start(
                oh_sb[:].rearrange("ql (qc p) -> ql qc p", p=P),
                oh[:].rearrange("qc ql p -> ql qc p"),
            )

            # L2 stream, 2 chunks per image for pipelining
            t2s = []
            for b in range(B_LOC):
                t2 = xtp.tile([128, NCH2 * HW2], fr, tag=f"t2_{b}", name=f"t2_{b}")
                v3 = t2[:].rearrange("cp (cc hw) -> cp cc hw", hw=HW2)
                s3 = feats[2][b].rearrange("(cc cp) h w -> cp cc (h w)", cp=128)
                eng = nc.sync if b == 0 else nc.scalar
                for c2 in range(2):
                    eng.dma_start(v3[:, 4 * c2 : 4 * c2 + 4], s3[:, 4 * c2 : 4 * c2 + 4])
                t2s.append(t2)

            # --- gather jobs: (level, patch, q), L1 first, sorted by q ---
            jobs = []
            for l in (1, 0):
                q = np.asarray(idx_vals[l], dtype=np.int64)
                for p_id in np.argsort(q, kind="stable"):
                    jobs.append((l, int(p_id), int(q[p_id])))

            # greedy cost-balanced assignment to queues
            load = dict(BASE)
            if not USE_DVE_DMA:
                load.pop("vector", None)
            q_jobs = {qn: [] for qn in qnames}
            for job in jobs:
                l = job[0]
                best = min(qnames, key=lambda qn: load[qn] + COST[qn][l])
                q_jobs[best].append(job)
                load[best] += COST[best][l]

            srcs, dsts = {}, {}
            for l in (0, 1):
                srcs[l] = feats[l][:].rearrange(
                    "b (ch cp) h w -> cp (b ch) (h w)", cp=128
                )
                dsts[l] = xts[l][:].rearrange("c (bc pp) -> c bc pp", pp=P)

            # --- L2 compute as a unit list, to be woven between gathers ---
            l2_units = []
            h2s = [
                xtp.tile([128, 2 * HW2], fr, tag=f"h2_{b}", name=f"h2_{b}")
                for b in range(B_LOC)
            ]
            py2_sbs = [
                consts.tile([128, NCD], f32, tag=f"py2sb_{b}", name=f"py2sb_{b}")
                for b in range(B_LOC)
            ]

            def mk_layer1(b, half):
                def emit():
                    gs = [
                        psum1.tile([128, 512], f32, tag=f"g{qn}", name=f"g{qn}")
                        for qn in range(2)
                    ]
                    for cc in range(NCH2):
                        o = cc * NCD + half * 128
                        for qn in range(2):
                            nc.tensor.matmul(
                                gs[qn][:],
                                w1_sb[2][:, o : o + 128],
                                t2s[b][
                                    :, cc * HW2 + qn * 512 : cc * HW2 + qn * 512 + 512
                                ],
                                start=(cc == 0),
                                stop=False,
                            )
                    for qn in range(2):
                        nc.tensor.matmul(
                            gs[qn][:],
                            b1_sb[2][0:1, half * 128 : half * 128 + 128],
                            ones[0:1, 0:512],
                            start=False,
                            stop=True,
                        )
                        nc.scalar.activation(
                            h2s[b][
                                :,
                                (half * 2 + qn) * 512 : (half * 2 + qn) * 512 + 512,
                            ],
                            gs[qn][:],
                            AF.Relu,
                        )

                return emit

            py2_cur = [None]

            def mk_layer2(b, qc):
                def emit():
                    if qc == 0:
                        py2_cur[0] = psum.tile([128, NCD], f32, tag="py", name="py2")
                    py2 = py2_cur[0]
                    k = psum.tile([128, NCD], f32, tag="k", name="k")
                    for half in range(2):
                        o = (half * 2 + qc // 4) * 512 + (qc % 4) * 128
                        nc.tensor.matmul(
                            k[:],
                            h2s[b][:, o : o + 128],
                            w2_sb[2][:, half * NCD : (half + 1) * NCD],
                            start=(half == 0),
                            stop=False,
                        )
                    nc.tensor.matmul(
                        k[:],
                        ones[0:1, 0:128],
                        b2_sb[2][0:1, :],
                        start=False,
                        stop=True,
                    )
                    ksb = work.tile([128, NCD], fr, tag="ksb", name="ksb")
                    nc.scalar.copy(ksb[:], k[:])
                    nc.tensor.matmul(
                        py2[:],
                        oh_sb[:, qc * P : (qc + 1) * P],
                        ksb[:],
                        start=(qc == 0),
                        stop=(qc == QC2 - 1),
                    )
                    if qc == QC2 - 1:
                        # off PSUM right away so the bank is reusable early
                        nc.scalar.copy(py2_sbs[b][:], py2[:])

                return emit

            for b in range(B_LOC):
                for half in range(2):
                    l2_units.append(mk_layer1(b, half))
                for qc in range(QC2):
                    l2_units.append(mk_layer2(b, qc))

            # --- weave: emit gather DMAs round-robin, L2 units in between ---
            with nc.allow_non_contiguous_dma("sparse patch gather"):
                n_rounds = max(len(v) for v in q_jobs.values())
                n_units = len(l2_units)
                emitted_units = 0
                for r in range(n_rounds):
                    for qn in qnames:
                        if r < len(q_jobs[qn]):
                            l, p_id, q = q_jobs[qn][r]
                            QS[qn].dma_start(
                                dsts[l][:, :, p_id], srcs[l][:, :, q]
                            )
                    want = (r + 1) * n_units // n_rounds
                    while emitted_units < want:
                        l2_units[emitted_units]()
                        emitted_units += 1
                while emitted_units < n_units:
                    l2_units[emitted_units]()
                    emitted_units += 1

            # --- L0/L1 MLPs (fp32r), L1 first; relus on DVE ---
            pys = {}
            for l in (1, 0):
                C, H = LEVELS[l]
                n_ch = C // 128
                x4 = xts[l][:].rearrange("c (b ch p) -> c ch b p", b=B_LOC, p=P)
                hts = []
                for half in range(2):
                    ph = psum1.tile([128, B_LOC * P], f32, tag="ph", name="ph")
                    for ch in range(n_ch):
                        o = ch * NCD + half * 128
                        nc.tensor.matmul(
                            ph[:],
                            w1_sb[l][:, o : o + 128],
                            x4[:, ch],
                            start=(ch == 0),
                            stop=False,
                        )
                    nc.tensor.matmul(
                        ph[:],
                        b1_sb[l][0:1, half * 128 : half * 128 + 128],
                        ones[0:1, 0 : B_LOC * P],
                        start=False,
                        stop=True,
                    )
                    ht = work.tile([128, B_LOC * P], fr, tag="ht", name="ht")
                    nc.vector.tensor_relu(ht[:], ph[:])
                    hts.append(ht)

                for b in range(B_LOC):
                    py = psum.tile([128, NCD], f32, tag="py", name="py")
                    for half in range(2):
                        nc.tensor.matmul(
                            py[:],
                            hts[half][:, b * P : (b + 1) * P],
                            w2_sb[l][:, half * NCD : (half + 1) * NCD],
                            start=(half == 0),
                            stop=False,
                        )
                    nc.tensor.matmul(
                        py[:],
                        ones[0:1, 0:P],
                        b2_sb[l][0:1, :],
                        start=False,
                        stop=True,
                    )
                    pys[(l, b)] = py

            # --- norms + stores; L1/L0 first (frees py slots), L2 from SBUF ---
            store_engs = [nc.sync, nc.scalar, nc.gpsimd]
            order = [(1, 0), (1, 1), (0, 0), (0, 1), (2, 0), (2, 1)]
            for si, (l, b) in enumerate(order):
                src = pys[(l, b)] if l < 2 else py2_sbs[b]
                _norm_and_store(nc, work, AF, f32, src, out, l, b, store_engs[si % 3])

    nc.compile()
    return nc


def _norm_and_store(nc, work, AF, f32, py, out, l, b, eng):
    sq = work.tile([128, NCD], f32, tag="sq", name="sq")
    ssq = work.tile([128, 1], f32, tag="ssq", name="ssq")
    nc.scalar.activation(sq[:], py[:], AF.Square, accum_out=ssq[:])
    nrm = work.tile([128, 1], f32, tag="nrm", name="nrm")
    nc.scalar.sqrt(nrm[:], ssq[:])
    nrm2 = work.tile([128, 1], f32, tag="nrm2", name="nrm2")
    nc.vector.tensor_scalar_add(nrm2[:], nrm[:], EPS)
    inv = work.tile([128, 1], f32, tag="inv", name="inv")
    nc.vector.reciprocal(inv[:], nrm2[:])
    yo = work.tile([128, NCD], f32, tag="yo", name="yo")
    nc.scalar.mul(yo[:], py[:], inv[:])
    eng.dma_start(out[l, b], yo[:])


def _run(inputs, trace=False):
    from concourse.bass_utils import run_bass_kernel_spmd

    feats = [
        np.ascontiguousarray(np.asarray(inputs[f"feat{l}"], dtype=np.float32))
        for l in range(3)
    ]
    idxs = [np.asarray(inputs[f"idx{l}"]).astype(np.int64) for l in range(3)]
    nc = _build(idxs)

    oh2 = np.zeros((8, 128, P), np.float32)
    for p, q in enumerate(idxs[2]):
        oh2[int(q) // 128, int(q) % 128, p] = 1.0

    in_maps = []
    for c in range(N_CORES):
        m = {"oh2": oh2}
        for l in range(3):
            m[f"feat{l}"] = feats[l][c * B_LOC : (c + 1) * B_LOC]
            m[f"w1_{l}"] = np.asarray(inputs[f"w1_{l}"], dtype=np.float32)
            m[f"b1_{l}"] = np.asarray(inputs[f"b1_{l}"], dtype=np.float32)
            m[f"w2_{l}"] = np.asarray(inputs[f"w2_{l}"], dtype=np.float32)
            m[f"b2_{l}"] = np.asarray(inputs[f"b2_{l}"], dtype=np.float32)
        in_maps.append(m)

    res = run_bass_kernel_spmd(
        nc, in_maps, core_ids=list(range(N_CORES)), trace=trace
    )
    full = np.concatenate([r["out"] for r in res.results], axis=1)
    return full.astype(np.float32), res


def kernel(**inputs) -> np.ndarray:
    out, _ = _run(inputs, trace=False)
    return out


# revision 22
# speedup vs baseline: 1.0406x; 1.0406x over previous
"""NPMLPHead sampling kernel for Trainium2 (Bass/Tile), SPMD over 8 cores.

Strategy (data-parallel over batch, per sharding hint): B=16 -> 2 images/core,
full (tiny) MLP weights replicated per core.

v2 redesign around the measured bottleneck (per-dma_start sequencer time on the
DMA-issue queues, ~0.9-1.6us each for the 256 patch gathers):
  - L0/L1 (sparse: 128 of 16K/4K positions): gather patch column-vectors with
    one strided DMA per patch ([cp, (b ch)] elements; land transposed in SBUF).
    The 256 gather DMAs are cost-balanced across FOUR issue queues: both HWDGE
    rings (sync/scalar), the SWDGE ring (gpsimd), and the DVE sequencer (HWDGE
    policy extended -- the RTL is engine-agnostic).  Patches are issued in
    sorted-position order for HBM row locality.
  - L2 (dense-ish: 128 of 1024): stream the whole shard and use associativity
    y = S^T ((relu(W1^T T))^T W2); layer-1/2 over ALL positions keeps the
    contraction dim on partitions in the native [C, HW] layout, and the
    one-hot select is a single PE matmul chain.  The whole L2 compute is
    WOVEN between the gather DMAs (PE runs it under the gather phase; its
    relu/copy work is interleaved into the scalar queue's instruction stream
    so nothing stalls).
  - L1 gathers go first so the L1 MLP (the bigger tail) overlaps the L0
    gather phase; elementwise tails ride DVE.
  - All matmuls in float32r (single-pass fp32; ~tf32 rounding).
"""

import sys

sys.path.insert(0, "/opt/trn_rl_repo")

import numpy as np

B = 16
N_CORES = 8
B_LOC = B // N_CORES  # 2
P = 128  # NUM_PATCHES
NCD = 256  # MLP width
LEVELS = [(256, 128), (512, 64), (1024, 32)]  # (C, H) per level
EPS = 1e-7

USE_DVE_DMA = False  # compiler supports only the two HWDGE queues + SWDGE

# per-queue per-job issue-cost model (ns), used only for load balancing
COST = {
    "sync": {1: 1106, 0: 865},
    "scalar": {1: 1106, 0: 865},
    "vector": {1: 1146, 0: 905},
    "gpsimd": {1: 1636, 0: 1462},
}
BASE = {"sync": 6000, "scalar": 23800, "vector": 2500, "gpsimd": 0}


def _build(idx_vals):
    import concourse.bass as bass
    import concourse.tile as tile
    from concourse import bacc, mybir

    f32 = mybir.dt.float32
    fr = mybir.dt.float32r
    AF = mybir.ActivationFunctionType

    nc = bacc.Bacc(None)
    if USE_DVE_DMA:
        nc.hwdge_engines.add(mybir.EngineType.DVE)

    feats, w1s, b1s, w2s, b2s = [], [], [], [], []
    for l, (C, H) in enumerate(LEVELS):
        feats.append(
            nc.dram_tensor(f"feat{l}", [B_LOC, C, H, H], fr, kind="ExternalInput")
        )
        w1s.append(nc.dram_tensor(f"w1_{l}", [C, NCD], fr, kind="ExternalInput"))
        b1s.append(nc.dram_tensor(f"b1_{l}", [NCD], fr, kind="ExternalInput"))
        w2s.append(nc.dram_tensor(f"w2_{l}", [NCD, NCD], fr, kind="ExternalInput"))
        b2s.append(nc.dram_tensor(f"b2_{l}", [NCD], fr, kind="ExternalInput"))
    oh = nc.dram_tensor("oh2", [8, 128, P], fr, kind="ExternalInput")
    out = nc.dram_tensor("out", [3, B_LOC, P, NCD], f32, kind="ExternalOutput")

    C2, H2 = LEVELS[2]
    HW2 = H2 * H2  # 1024
    NCH2 = C2 // 128  # 8
    QC2 = HW2 // 128  # 8 q-chunks

    with tile.TileContext(nc) as tc:
        with (
            tc.tile_pool(name="consts", bufs=1) as consts,
            tc.tile_pool(name="xt", bufs=1) as xtp,
            tc.tile_pool(name="work", bufs=4) as work,
            tc.tile_pool(name="psum", bufs=2, space=bass.MemorySpace.PSUM) as psum,
            tc.tile_pool(name="psum1", bufs=1, space=bass.MemorySpace.PSUM) as psum1,
        ):
            QS = {
                "sync": nc.sync,
                "scalar": nc.scalar,
                "vector": nc.vector,
                "gpsimd": nc.gpsimd,
            }
            qnames = ["sync", "scalar", "gpsimd"] + (
                ["vector"] if USE_DVE_DMA else []
            )

            ones_f = consts.tile([1, 512], f32, tag="ones_f")
            nc.vector.memset(ones_f[:], 1.0)
            ones = consts.tile([1, 512], fr, tag="ones")
            nc.vector.tensor_copy(ones[:], ones_f[:])

            # --- early loads, alternating the two HWDGE rings ---
            w1_sb, w2_sb, b1_sb, b2_sb, xts = [], [], [], [], []
            for l, (C, H) in enumerate(LEVELS):
                n_ch = C // 128
                t = consts.tile([128, n_ch * NCD], fr, tag=f"w1_{l}", name=f"w1sb{l}")
                (nc.scalar if l == 2 else nc.sync).dma_start(
                    t[:].rearrange("cp (ch n) -> cp ch n", n=NCD),
                    w1s[l][:].rearrange("(ch cp) n -> cp ch n", cp=128),
                )
                w1_sb.append(t)
                t = consts.tile([128, 2 * NCD], fr, tag=f"w2_{l}", name=f"w2sb{l}")
                (nc.scalar if l == 2 else nc.sync).dma_start(
                    t[:].rearrange("cp (ch n) -> cp ch n", n=NCD),
                    w2s[l][:].rearrange("(ch cp) n -> cp ch n", cp=128),
                )
                w2_sb.append(t)
                t = consts.tile([1, NCD], fr, tag=f"b1_{l}", name=f"b1sb{l}")
                nc.sync.dma_start(t[:], b1s[l][:].rearrange("(o n) -> o n", o=1))
                b1_sb.append(t)
                t = consts.tile([1, NCD], fr, tag=f"b2_{l}", name=f"b2sb{l}")
                nc.sync.dma_start(t[:], b2s[l][:].rearrange("(o n) -> o n", o=1))
                b2_sb.append(t)
                if l < 2:
                    xts.append(
                        xtp.tile(
                            [128, B_LOC * n_ch * P], fr, tag=f"xt_{l}", name=f"xt{l}"
                        )
                    )

            oh_sb = consts.tile([128, QC2 * P], fr, tag="oh2")
            nc.scalar.dma_start(
                oh_sb[:].rearrange("ql (qc p) -> ql qc p", p=P),
                oh[:].rearrange("qc ql p -> ql qc p"),
            )

            # L2 stream, 2 chunks per image for pipelining
            t2s = []
            for b in range(B_LOC):
                t2 = xtp.tile([128, NCH2 * HW2], fr, tag=f"t2_{b}", name=f"t2_{b}")
                v3 = t2[:].rearrange("cp (cc hw) -> cp cc hw", hw=HW2)
                s3 = feats[2][b].rearrange("(cc cp) h w -> cp cc (h w)", cp=128)
                eng = nc.sync if b == 0 else nc.scalar
                for c2 in range(2):
                    eng.dma_start(v3[:, 4 * c2 : 4 * c2 + 4], s3[:, 4 * c2 : 4 * c2 + 4])
                t2s.append(t2)

            # --- gather jobs: (level, patch, q), L1 first, sorted by q ---
            jobs = []
            for l in (1, 0):
                q = np.asarray(idx_vals[l], dtype=np.int64)
                for p_id in np.argsort(q, kind="stable"):
                    jobs.append((l, int(p_id), int(q[p_id])))

            # greedy cost-balanced assignment to queues
            load = dict(BASE)
            if not USE_DVE_DMA:
                load.pop("vector", None)
            q_jobs = {qn: [] for qn in qnames}
            for job in jobs:
                l = job[0]
                best = min(qnames, key=lambda qn: load[qn] + COST[qn][l])
                q_jobs[best].append(job)
                load[best] += COST[best][l]

            srcs, dsts = {}, {}
            for l in (0, 1):
                srcs[l] = feats[l][:].rearrange(
                    "b (ch cp) h w -> cp (b ch) (h w)", cp=128
                )
                dsts[l] = xts[l][:].rearrange("c (bc pp) -> c bc pp", pp=P)

            # --- L2 compute as a unit list, to be woven between gathers ---
            l2_units = []
            h2s = [
                xtp.tile([128, 2 * HW2], fr, tag=f"h2_{b}", name=f"h2_{b}")
                for b in range(B_LOC)
            ]
            py2_sbs = [
                consts.tile([128, NCD], f32, tag=f"py2sb_{b}", name=f"py2sb_{b}")
                for b in range(B_LOC)
            ]

            def mk_layer1(b, half):
                def emit():
                    gs = [
                        psum1.tile([128, 512], f32, tag=f"g{qn}", name=f"g{qn}")
                        for qn in range(2)
                    ]
                    for cc in range(NCH2):
                        o = cc * NCD + half * 128
                        for qn in range(2):
                            nc.tensor.matmul(
                                gs[qn][:],
                                w1_sb[2][:, o : o + 128],
                                t2s[b][
                                    :, cc * HW2 + qn * 512 : cc * HW2 + qn * 512 + 512
                                ],
                                start=(cc == 0),
                                stop=False,
                            )
                    for qn in range(2):
                        nc.tensor.matmul(
                            gs[qn][:],
                            b1_sb[2][0:1, half * 128 : half * 128 + 128],
                            ones[0:1, 0:512],
                            start=False,
                            stop=True,
                        )
                        nc.scalar.activation(
                            h2s[b][
                                :,
                                (half * 2 + qn) * 512 : (half * 2 + qn) * 512 + 512,
                            ],
                            gs[qn][:],
                            AF.Relu,
                        )

                return emit

            py2_cur = [None]

            def mk_layer2(b, qc):
                def emit():
                    if qc == 0:
                        py2_cur[0] = psum.tile([128, NCD], f32, tag="py", name="py2")
                    py2 = py2_cur[0]
                    k = psum.tile([128, NCD], f32, tag="k", name="k")
                    for half in range(2):
                        o = (half * 2 + qc // 4) * 512 + (qc % 4) * 128
                        nc.tensor.matmul(
                            k[:],
                            h2s[b][:, o : o + 128],
                            w2_sb[2][:, half * NCD : (half + 1) * NCD],
                            start=(half == 0),
                            stop=False,
                        )
                    nc.tensor.matmul(
                        k[:],
                        ones[0:1, 0:128],
                        b2_sb[2][0:1, :],
                        start=False,
                        stop=True,
                    )
                    ksb = work.tile([128, NCD], fr, tag="ksb", name="ksb")
                    nc.scalar.copy(ksb[:], k[:])
                    nc.tensor.matmul(
                        py2[:],
                        oh_sb[:, qc * P : (qc + 1) * P],
                        ksb[:],
                        start=(qc == 0),
                        stop=(qc == QC2 - 1),
                    )
                    if qc == QC2 - 1:
                        # off PSUM right away so the bank is reusable early
                        nc.scalar.copy(py2_sbs[b][:], py2[:])

                return emit

            for b in range(B_LOC):
                for half in range(2):
                    l2_units.append(mk_layer1(b, half))
                for qc in range(QC2):
                    l2_units.append(mk_layer2(b, qc))

            # --- weave: emit gather DMAs round-robin, L2 units in between ---
            with nc.allow_non_contiguous_dma("sparse patch gather"):
                n_rounds = max(len(v) for v in q_jobs.values())
                n_units = len(l2_units)
                emitted_units = 0
                for r in range(n_rounds):
                    for qn in qnames:
                        if r < len(q_jobs[qn]):
                            l, p_id, q = q_jobs[qn][r]
                            QS[qn].dma_start(
                                dsts[l][:, :, p_id],
                                srcs[l][:, :, q],
                                single_packet=True,
                            )
                    want = (r + 1) * n_units // n_rounds
                    while emitted_units < want:
                        l2_units[emitted_units]()
                        emitted_units += 1
                while emitted_units < n_units:
                    l2_units[emitted_units]()
                    emitted_units += 1

            # --- L0/L1 MLPs (fp32r), L1 first; relus on DVE ---
            pys = {}
            for l in (1, 0):
                C, H = LEVELS[l]
                n_ch = C // 128
                x4 = xts[l][:].rearrange("c (b ch p) -> c ch b p", b=B_LOC, p=P)
                hts = []
                for half in range(2):
                    ph = psum1.tile([128, B_LOC * P], f32, tag="ph", name="ph")
                    for ch in range(n_ch):
                        o = ch * NCD + half * 128
                        nc.tensor.matmul(
                            ph[:],
                            w1_sb[l][:, o : o + 128],
                            x4[:, ch],
                            start=(ch == 0),
                            stop=False,
                        )
                    nc.tensor.matmul(
                        ph[:],
                        b1_sb[l][0:1, half * 128 : half * 128 + 128],
                        ones[0:1, 0 : B_LOC * P],
                        start=False,
                        stop=True,
                    )
                    ht = work.tile([128, B_LOC * P], fr, tag="ht", name="ht")
                    nc.vector.tensor_relu(ht[:], ph[:])
                    hts.append(ht)

                for b in range(B_LOC):
                    py = psum.tile([128, NCD], f32, tag="py", name="py")
                    for half in range(2):
                        nc.tensor.matmul(
                            py[:],
                            hts[half][:, b * P : (b + 1) * P],
                            w2_sb[l][:, half * NCD : (half + 1) * NCD],
                            start=(half == 0),
                            stop=False,
                        )
                    nc.tensor.matmul(
                        py[:],
                        ones[0:1, 0:P],
                        b2_sb[l][0:1, :],
                        start=False,
                        stop=True,
                    )
                    pys[(l, b)] = py

            # --- norms + stores; L1/L0 first (frees py slots), L2 from SBUF ---
            store_engs = [nc.sync, nc.scalar, nc.gpsimd]
            order = [(1, 0), (1, 1), (0, 0), (0, 1), (2, 0), (2, 1)]
            for si, (l, b) in enumerate(order):
                src = pys[(l, b)] if l < 2 else py2_sbs[b]
                _norm_and_store(nc, work, AF, f32, src, out, l, b, store_engs[si % 3])

    nc.compile()
    return nc


def _norm_and_store(nc, work, AF, f32, py, out, l, b, eng):
    sq = work.tile([128, NCD], f32, tag="sq", name="sq")
    ssq = work.tile([128, 1], f32, tag="ssq", name="ssq")
    nc.scalar.activation(sq[:], py[:], AF.Square, accum_out=ssq[:])
    nrm = work.tile([128, 1], f32, tag="nrm", name="nrm")
    nc.scalar.sqrt(nrm[:], ssq[:])
    nrm2 = work.tile([128, 1], f32, tag="nrm2", name="nrm2")
    nc.vector.tensor_scalar_add(nrm2[:], nrm[:], EPS)
    inv = work.tile([128, 1], f32, tag="inv", name="inv")
    nc.vector.reciprocal(inv[:], nrm2[:])
    yo = work.tile([128, NCD], f32, tag="yo", name="yo")
    nc.scalar.mul(yo[:], py[:], inv[:])
    eng.dma_start(out[l, b], yo[:])


def _run(inputs, trace=False):
    from concourse.bass_utils import run_bass_kernel_spmd

    feats = [
        np.ascontiguousarray(np.asarray(inputs[f"feat{l}"], dtype=np.float32))
        for l in range(3)
    ]
    idxs = [np.asarray(inputs[f"idx{l}"]).astype(np.int64) for l in range(3)]
    nc = _build(idxs)

    oh2 = np.zeros((8, 128, P), np.float32)
    for p, q in enumerate(idxs[2]):
        oh2[int(q) // 128, int(q) % 128, p] = 1.0

    in_maps = []
    for c in range(N_CORES):
        m = {"oh2": oh2}
        for l in range(3):
            m[f"feat{l}"] = feats[l][c * B_LOC : (c + 1) * B_LOC]
            m[f"w1_{l}"] = np.asarray(inputs[f"w1_{l}"], dtype=np.float32)
            m[f"b1_{l}"] = np.asarray(inputs[f"b1_{l}"], dtype=np.float32)
            m[f"w2_{l}"] = np.asarray(inputs[f"w2_{l}"], dtype=np.float32)
            m[f"b2_{l}"] = np.asarray(inputs[f"b2_{l}"], dtype=np.float32)
        in_maps.append(m)

    res = run_bass_kernel_spmd(
        nc, in_maps, core_ids=list(range(N_CORES)), trace=trace
    )
    full = np.concatenate([r["out"] for r in res.results], axis=1)
    return full.astype(np.float32), res


def kernel(**inputs) -> np.ndarray:
    out, _ = _run(inputs, trace=False)
    return out


# revision 25
# speedup vs baseline: 1.0680x; 1.0264x over previous
"""NPMLPHead sampling kernel for Trainium2 (Bass/Tile), SPMD over 8 cores.

Strategy (data-parallel over batch, per sharding hint): B=16 -> 2 images/core,
full (tiny) MLP weights replicated per core.

v2 redesign around the measured bottleneck (per-dma_start sequencer time on the
DMA-issue queues, ~0.9-1.6us each for the 256 patch gathers):
  - L0/L1 (sparse: 128 of 16K/4K positions): gather patch column-vectors with
    one strided DMA per patch ([cp, (b ch)] elements; land transposed in SBUF).
    The 256 gather DMAs are cost-balanced across FOUR issue queues: both HWDGE
    rings (sync/scalar), the SWDGE ring (gpsimd), and the DVE sequencer (HWDGE
    policy extended -- the RTL is engine-agnostic).  Patches are issued in
    sorted-position order for HBM row locality.
  - L2 (dense-ish: 128 of 1024): stream the whole shard and use associativity
    y = S^T ((relu(W1^T T))^T W2); layer-1/2 over ALL positions keeps the
    contraction dim on partitions in the native [C, HW] layout, and the
    one-hot select is a single PE matmul chain.  The whole L2 compute is
    WOVEN between the gather DMAs (PE runs it under the gather phase; its
    relu/copy work is interleaved into the scalar queue's instruction stream
    so nothing stalls).
  - L1 gathers go first so the L1 MLP (the bigger tail) overlaps the L0
    gather phase; elementwise tails ride DVE.
  - All matmuls in float32r (single-pass fp32; ~tf32 rounding).
"""

import sys

sys.path.insert(0, "/opt/trn_rl_repo")

import numpy as np

B = 16
N_CORES = 8
B_LOC = B // N_CORES  # 2
P = 128  # NUM_PATCHES
NCD = 256  # MLP width
LEVELS = [(256, 128), (512, 64), (1024, 32)]  # (C, H) per level
EPS = 1e-7

USE_DVE_DMA = False  # compiler supports only the two HWDGE queues + SWDGE

# per-queue per-job issue-cost model (ns), used only for load balancing
COST = {
    "sync": {1: 1106, 0: 865},
    "scalar": {1: 1106, 0: 865},
    "vector": {1: 1146, 0: 905},
    "gpsimd": {1: 1636, 0: 1462},
}
BASE = {"sync": 6000, "scalar": 23800, "vector": 2500, "gpsimd": 0}


def _build(idx_vals):
    import concourse.bass as bass
    import concourse.tile as tile
    from concourse import bacc, mybir

    f32 = mybir.dt.float32
    fr = mybir.dt.float32r
    AF = mybir.ActivationFunctionType

    nc = bacc.Bacc(None)
    if USE_DVE_DMA:
        nc.hwdge_engines.add(mybir.EngineType.DVE)

    feats, w1s, b1s, w2s, b2s = [], [], [], [], []
    for l, (C, H) in enumerate(LEVELS):
        feats.append(
            nc.dram_tensor(f"feat{l}", [B_LOC, C, H, H], fr, kind="ExternalInput")
        )
        w1s.append(nc.dram_tensor(f"w1_{l}", [C, NCD], fr, kind="ExternalInput"))
        b1s.append(nc.dram_tensor(f"b1_{l}", [NCD], fr, kind="ExternalInput"))
        w2s.append(nc.dram_tensor(f"w2_{l}", [NCD, NCD], fr, kind="ExternalInput"))
        b2s.append(nc.dram_tensor(f"b2_{l}", [NCD], fr, kind="ExternalInput"))
    oh = nc.dram_tensor("oh2", [8, 128, P], fr, kind="ExternalInput")
    out = nc.dram_tensor("out", [3, B_LOC, P, NCD], f32, kind="ExternalOutput")

    C2, H2 = LEVELS[2]
    HW2 = H2 * H2  # 1024
    NCH2 = C2 // 128  # 8
    QC2 = HW2 // 128  # 8 q-chunks

    with tile.TileContext(nc) as tc:
        with (
            tc.tile_pool(name="consts", bufs=1) as consts,
            tc.tile_pool(name="xt", bufs=1) as xtp,
            tc.tile_pool(name="work", bufs=4) as work,
            tc.tile_pool(name="psum", bufs=2, space=bass.MemorySpace.PSUM) as psum,
            tc.tile_pool(name="psum1", bufs=1, space=bass.MemorySpace.PSUM) as psum1,
        ):
            QS = {
                "sync": nc.sync,
                "scalar": nc.scalar,
                "vector": nc.vector,
                "gpsimd": nc.gpsimd,
            }
            qnames = ["sync", "scalar", "gpsimd"] + (
                ["vector"] if USE_DVE_DMA else []
            )

            ones_f = consts.tile([1, 512], f32, tag="ones_f")
            nc.vector.memset(ones_f[:], 1.0)
            ones = consts.tile([1, 512], fr, tag="ones")
            nc.vector.tensor_copy(ones[:], ones_f[:])

            # --- early loads, alternating the two HWDGE rings ---
            w1_sb, w2_sb, b1_sb, b2_sb, xts = [], [], [], [], []
            for l, (C, H) in enumerate(LEVELS):
                n_ch = C // 128
                t = consts.tile([128, n_ch * NCD], fr, tag=f"w1_{l}", name=f"w1sb{l}")
                (nc.scalar if l == 2 else nc.sync).dma_start(
                    t[:].rearrange("cp (ch n) -> cp ch n", n=NCD),
                    w1s[l][:].rearrange("(ch cp) n -> cp ch n", cp=128),
                )
                w1_sb.append(t)
                t = consts.tile([128, 2 * NCD], fr, tag=f"w2_{l}", name=f"w2sb{l}")
                (nc.scalar if l == 2 else nc.sync).dma_start(
                    t[:].rearrange("cp (ch n) -> cp ch n", n=NCD),
                    w2s[l][:].rearrange("(ch cp) n -> cp ch n", cp=128),
                )
                w2_sb.append(t)
                t = consts.tile([1, NCD], fr, tag=f"b1_{l}", name=f"b1sb{l}")
                nc.sync.dma_start(t[:], b1s[l][:].rearrange("(o n) -> o n", o=1))
                b1_sb.append(t)
                t = consts.tile([1, NCD], fr, tag=f"b2_{l}", name=f"b2sb{l}")
                nc.sync.dma_start(t[:], b2s[l][:].rearrange("(o n) -> o n", o=1))
                b2_sb.append(t)
                if l < 2:
                    xts.append(
                        xtp.tile(
                            [128, B_LOC * n_ch * P], fr, tag=f"xt_{l}", name=f"xt{l}"
                        )
                    )

            oh_sb = consts.tile([128, QC2 * P], fr, tag="oh2")
            nc.scalar.dma_start(
                oh_sb[:].rearrange("ql (qc p) -> ql qc p", p=P),
                oh[:].rearrange("qc ql p -> ql qc p"),
            )

            # L2 stream, 2 chunks per image for pipelining
            t2s = []
            for b in range(B_LOC):
                t2 = xtp.tile([128, NCH2 * HW2], fr, tag=f"t2_{b}", name=f"t2_{b}")
                v3 = t2[:].rearrange("cp (cc hw) -> cp cc hw", hw=HW2)
                s3 = feats[2][b].rearrange("(cc cp) h w -> cp cc (h w)", cp=128)
                eng = nc.sync if b == 0 else nc.scalar
                for c2 in range(2):
                    eng.dma_start(v3[:, 4 * c2 : 4 * c2 + 4], s3[:, 4 * c2 : 4 * c2 + 4])
                t2s.append(t2)

            # --- gather jobs: (level, patch, q), L1 first, sorted by q ---
            jobs = []
            for l in (1, 0):
                q = np.asarray(idx_vals[l], dtype=np.int64)
                for p_id in np.argsort(q, kind="stable"):
                    jobs.append((l, int(p_id), int(q[p_id])))

            # greedy cost-balanced assignment to queues
            load = dict(BASE)
            if not USE_DVE_DMA:
                load.pop("vector", None)
            q_jobs = {qn: [] for qn in qnames}
            for job in jobs:
                l = job[0]
                best = min(qnames, key=lambda qn: load[qn] + COST[qn][l])
                q_jobs[best].append(job)
                load[best] += COST[best][l]

            srcs, dsts = {}, {}
            for l in (0, 1):
                srcs[l] = feats[l][:].rearrange(
                    "b (ch cp) h w -> cp (b ch) (h w)", cp=128
                )
                dsts[l] = xts[l][:].rearrange("c (bc pp) -> c bc pp", pp=P)

            # --- L2 compute as a unit list, to be woven between gathers ---
            l2_units = []
            h2s = [
                xtp.tile([128, 2 * HW2], fr, tag=f"h2_{b}", name=f"h2_{b}")
                for b in range(B_LOC)
            ]
            py2_sbs = [
                consts.tile([128, NCD], f32, tag=f"py2sb_{b}", name=f"py2sb_{b}")
                for b in range(B_LOC)
            ]

            def mk_layer1(b, half):
                def emit():
                    gs = [
                        psum1.tile([128, 512], f32, tag=f"g{qn}", name=f"g{qn}")
                        for qn in range(2)
                    ]
                    for cc in range(NCH2):
                        o = cc * NCD + half * 128
                        for qn in range(2):
                            nc.tensor.matmul(
                                gs[qn][:],
                                w1_sb[2][:, o : o + 128],
                                t2s[b][
                                    :, cc * HW2 + qn * 512 : cc * HW2 + qn * 512 + 512
                                ],
                                start=(cc == 0),
                                stop=False,
                            )
                    for qn in range(2):
                        nc.tensor.matmul(
                            gs[qn][:],
                            b1_sb[2][0:1, half * 128 : half * 128 + 128],
                            ones[0:1, 0:512],
                            start=False,
                            stop=True,
                        )
                        nc.scalar.activation(
                            h2s[b][
                                :,
                                (half * 2 + qn) * 512 : (half * 2 + qn) * 512 + 512,
                            ],
                            gs[qn][:],
                            AF.Relu,
                        )

                return emit

            py2_cur = [None]

            def mk_layer2(b, qc):
                def emit():
                    if qc == 0:
                        py2_cur[0] = psum.tile([128, NCD], f32, tag="py", name="py2")
                    py2 = py2_cur[0]
                    k = psum.tile([128, NCD], f32, tag="k", name="k")
                    for half in range(2):
                        o = (half * 2 + qc // 4) * 512 + (qc % 4) * 128
                        nc.tensor.matmul(
                            k[:],
                            h2s[b][:, o : o + 128],
                            w2_sb[2][:, half * NCD : (half + 1) * NCD],
                            start=(half == 0),
                            stop=False,
                        )
                    nc.tensor.matmul(
                        k[:],
                        ones[0:1, 0:128],
                        b2_sb[2][0:1, :],
                        start=False,
                        stop=True,
                    )
                    ksb = work.tile([128, NCD], fr, tag="ksb", name="ksb")
                    nc.scalar.copy(ksb[:], k[:])
                    nc.tensor.matmul(
                        py2[:],
                        oh_sb[:, qc * P : (qc + 1) * P],
                        ksb[:],
                        start=(qc == 0),
                        stop=(qc == QC2 - 1),
                    )
                    if qc == QC2 - 1:
                        # off PSUM right away so the bank is reusable early
                        nc.scalar.copy(py2_sbs[b][:], py2[:])

                return emit

            for b in range(B_LOC):
                for half in range(2):
                    l2_units.append(mk_layer1(b, half))
                for qc in range(QC2):
                    l2_units.append(mk_layer2(b, qc))

            # --- weave: emit gather DMAs round-robin, L2 units in between ---
            with nc.allow_non_contiguous_dma("sparse patch gather"):
                n_rounds = max(len(v) for v in q_jobs.values())
                n_units = len(l2_units)
                emitted_units = 0
                for r in range(n_rounds):
                    for qn in qnames:
                        if r < len(q_jobs[qn]):
                            l, p_id, q = q_jobs[qn][r]
                            QS[qn].dma_start(
                                dsts[l][:, :, p_id],
                                srcs[l][:, :, q],
                                single_packet=True,
                            )
                    want = (r + 1) * n_units // n_rounds
                    while emitted_units < want:
                        l2_units[emitted_units]()
                        emitted_units += 1
                while emitted_units < n_units:
                    l2_units[emitted_units]()
                    emitted_units += 1

            # --- L0/L1 MLPs (fp32r), L1 first; relus on DVE ---
            pys = {}
            for l in (1, 0):
                C, H = LEVELS[l]
                n_ch = C // 128
                x4 = xts[l][:].rearrange("c (b ch p) -> c ch b p", b=B_LOC, p=P)
                hts = []
                for half in range(2):
                    ph = psum1.tile([128, B_LOC * P], f32, tag="ph", name="ph")
                    for ch in range(n_ch):
                        o = ch * NCD + half * 128
                        nc.tensor.matmul(
                            ph[:],
                            w1_sb[l][:, o : o + 128],
                            x4[:, ch],
                            start=(ch == 0),
                            stop=False,
                        )
                    nc.tensor.matmul(
                        ph[:],
                        b1_sb[l][0:1, half * 128 : half * 128 + 128],
                        ones[0:1, 0 : B_LOC * P],
                        start=False,
                        stop=True,
                    )
                    ht = work.tile([128, B_LOC * P], fr, tag="ht", name="ht")
                    nc.vector.tensor_relu(ht[:], ph[:])
                    hts.append(ht)

                for b in range(B_LOC):
                    py = psum.tile([128, NCD], f32, tag="py", name="py")
                    for half in range(2):
                        nc.tensor.matmul(
                            py[:],
                            hts[half][:, b * P : (b + 1) * P],
                            w2_sb[l][:, half * NCD : (half + 1) * NCD],
                            start=(half == 0),
                            stop=False,
                        )
                    nc.tensor.matmul(
                        py[:],
                        ones[0:1, 0:P],
                        b2_sb[l][0:1, :],
                        start=False,
                        stop=True,
                    )
                    pys[(l, b)] = py

            # --- norms + stores; L1/L0 first (frees py slots), L2 from SBUF ---
            store_engs = [nc.sync, nc.scalar, nc.gpsimd]
            order = [(1, 0), (1, 1), (0, 0), (0, 1), (2, 0), (2, 1)]
            for si, (l, b) in enumerate(order):
                src = pys[(l, b)] if l < 2 else py2_sbs[b]
                _norm_and_store(nc, work, AF, f32, src, out, l, b, store_engs[si % 3])

    nc.compile()
    return nc


def _norm_and_store(nc, work, AF, f32, py, out, l, b, eng):
    sq = work.tile([128, NCD], f32, tag="sq", name="sq")
    ssq = work.tile([128, 1], f32, tag="ssq", name="ssq")
    nc.scalar.activation(sq[:], py[:], AF.Square, accum_out=ssq[:])
    nrm = work.tile([128, 1], f32, tag="nrm", name="nrm")
    nc.scalar.sqrt(nrm[:], ssq[:])
    nrm2 = work.tile([128, 1], f32, tag="nrm2", name="nrm2")
    nc.vector.tensor_scalar_add(nrm2[:], nrm[:], EPS)
    inv = work.tile([128, 1], f32, tag="inv", name="inv")
    nc.vector.reciprocal(inv[:], nrm2[:])
    yo = work.tile([128, NCD], f32, tag="yo", name="yo")
    nc.scalar.mul(yo[:], py[:], inv[:])
    eng.dma_start(out[l, b], yo[:])


def _run(inputs, trace=False):
    from concourse.bass_utils import run_bass_kernel_spmd

    feats = [
        np.ascontiguousarray(np.asarray(inputs[f"feat{l}"], dtype=np.float32))
        for l in range(3)
    ]
    idxs = [np.asarray(inputs[f"idx{l}"]).astype(np.int64) for l in range(3)]
    nc = _build(idxs)

    oh2 = np.zeros((8, 128, P), np.float32)
    for p, q in enumerate(idxs[2]):
        oh2[int(q) // 128, int(q) % 128, p] = 1.0

    in_maps = []
    for c in range(N_CORES):
        m = {"oh2": oh2}
        for l in range(3):
            m[f"feat{l}"] = feats[l][c * B_LOC : (c + 1) * B_LOC]
            m[f"w1_{l}"] = np.asarray(inputs[f"w1_{l}"], dtype=np.float32)
            m[f"b1_{l}"] = np.asarray(inputs[f"b1_{l}"], dtype=np.float32)
            m[f"w2_{l}"] = np.asarray(inputs[f"w2_{l}"], dtype=np.float32)
            m[f"b2_{l}"] = np.asarray(inputs[f"b2_{l}"], dtype=np.float32)
        in_maps.append(m)

    res = run_bass_kernel_spmd(
        nc, in_maps, core_ids=list(range(N_CORES)), trace=trace
    )
    full = np.concatenate([r["out"] for r in res.results], axis=1)
    return full.astype(np.float32), res


def kernel(**inputs) -> np.ndarray:
    out, _ = _run(inputs, trace=False)
    return out
